# revision 32
# baseline (speedup 1.0000x reference)
"""Quantum multi-head attention TRN2 kernel (self-contained).

Problem: x(4,2048,1024); qp=cos(x+theta) per-head(16x64); q/k/v = qp@W*+b*
(per-head shared 64x64 weights); full softmax attention; merge heads; @Wo+bo.

Sharding: 8 cores = (batch b, seq-half j).  Each core gets the full batch-b
sequence (rolled so its 1024 query rows come first) and computes attention for
all 16 heads over its query rows, plus the final out-projection.  No
collectives; host just concatenates core outputs.

Device algorithm per core:
  host prep: y = (x + theta + pi/2)/(2pi) (fp16, both layouts);
    G = Wq@Wk^T (k-projection folded into scores; the bk terms cancel
    under softmax shift-invariance, the bq term folds into u's bias);
    wvo = blockdiag(Wv)@Wo bf16; bob = bv-row@Wo + bo.
  phase A (trig ACT table):
    qpT8 = sin2pi-range-reduce(y) -> ACT Sin -> fp8  (transposed layout)
    qpn  = same in natural layout, [qp|1] fp8 chunk-pair tiles
    u = G @ qpT8 + bu  per pair via row-grouped fp8 matmuls ->
        ACT Identity+bias -> fp8 uT8
  phase B (exp ACT table), per query-half, per head:
    scores = qpT8^T uT8   plain fp8 matmuls (K=64, full rate)
    e = exp(scores/8)     ACT -> fp8 [128,1024] = [2 chunks x 512q]
    ctx+den = qpn^T e     fp8 DoubleRow (K=256 chunk pairs) psum [65,512]
    ctxT = ctx * (1/den)  bf16, deferred normalize (spread reciprocal)
    after each query-half: its out-projection (bf16) overlaps the other
    half's ACT-bound attention.
"""
import numpy as np
import ml_dtypes

import concourse.bass as bass
import concourse.mybir as mybir
import concourse.tile as tile
from concourse.bass_utils import run_bass_kernel_spmd

F32 = mybir.dt.float32
BF16 = mybir.dt.bfloat16
FP16 = mybir.dt.float16
F8 = mybir.dt.float8e4
I32 = mybir.dt.int32
nbf16 = ml_dtypes.bfloat16
nf8 = ml_dtypes.float8_e4m3
PI = float(np.pi)
MAGIC = 12582912.0  # 1.5 * 2**23: forces round in the DVE's f32 pipeline
A = mybir.AluOpType
AF = mybir.ActivationFunctionType
DRM = mybir.MatmulPerfMode.DoubleRow

B, S, E = 4, 2048, 1024
H, HD = 16, 64
SQ = 1024          # query rows per core
N_CORES = 8
TRACE = False
LAST_RES = None


def _split_multiwaits(nc):
    """This container's walrus supports ONE sync-wait per instruction; split
    extras onto single-wait no-ops on the same engine (program order keeps
    semantics)."""
    counter = 0
    for f in nc.m.functions:
        for bb in f.blocks:
            new_insts = []
            for inst in bb.instructions:
                si = inst.sync_info
                if si is not None and si.on_wait and len(si.on_wait) > 1:
                    waits = list(si.on_wait)
                    si.on_wait = [waits[-1]]
                    for w in waits[:-1]:
                        counter += 1
                        new_insts.append(mybir.InstNoOp(
                            name=f"splitw-{counter}",
                            engine=inst.engine,
                            sync_info=mybir.SyncInfo(on_wait=[w], on_update=[]),
                            bass_nofuse=True,
                        ))
                new_insts.append(inst)
            bb.instructions[:] = new_insts
    return counter


def _build():
    nc = bass.Bass("TRN2", target_bir_lowering=False, debug=False)

    xt = nc.dram_tensor("xt", [E, S], FP16, kind="ExternalInput")
    xn = nc.dram_tensor("xn", [S, E], FP16, kind="ExternalInput")
    g2 = nc.dram_tensor("g2", [128, HD], F8, kind="ExternalInput")
    wvo_in = nc.dram_tensor("wvo_in", [E, E], BF16, kind="ExternalInput")
    bu2 = nc.dram_tensor("bu2", [128, 1], F32, kind="ExternalInput")
    bob_r = nc.dram_tensor("bob_r", [1, E], F32, kind="ExternalInput")
    out = nc.dram_tensor("out", [SQ, E], F32, kind="ExternalOutput")

    with tile.TileContext(nc) as tc:
        with (
            tc.tile_pool(name="persist", bufs=1) as pp,
            tc.tile_pool(name="cosx", bufs=5) as cosx_pool,
        ):
            # ---- persistent consts
            g2_t = pp.tile([128, HD], F8, name="g2_t")
            nc.sync.dma_start(g2_t[:], g2.ap())
            bu2_t = pp.tile([128, 1], F32, name="bu2_t")
            nc.sync.dma_start(bu2_t[:], bu2.ap())
            bobc_t = pp.tile([128, E], F32, name="bobc_t")
            nc.sync.dma_start(bobc_t[:], bob_r.ap().broadcast_to([128, E]))

            # persistent big arrays
            qpn = [pp.tile([128, 2 * H * 65], F8, name=f"qpn_{m}")
                   for m in range(8)]
            qpT8 = [pp.tile([128, S], F8, name=f"qpT8_{t}") for t in range(8)]
            uT8 = [pp.tile([128, SQ], F8, name=f"uT8_{t}") for t in range(8)]
            ctxT = [pp.tile([128, SQ], BF16, name=f"ctxT_{t}") for t in range(8)]
            wvo = [pp.tile([128, E], BF16, name=f"wvo_{t}") for t in range(8)]
            for t in range(8):
                nc.sync.dma_start(wvo[t][:],
                                  wvo_in.ap()[128 * t:128 * t + 128, :])

            def build_qpn(m):
                # qpn chunk-pair m: [qp|1] fp8, ctx DoubleRow stationary
                qv = qpn[m][:].rearrange("p (two h c) -> p two h c",
                                         two=2, c=65)
                for half in range(2):
                    jn = 2 * m + half
                    xn_t = cosx_pool.tile([128, E], FP16,
                                          name=f"xn_{jn}", tag="xn_in")
                    nc.sync.dma_start(xn_t[:],
                                      xn.ap()[128 * jn:128 * jn + 128, :])
                    tn = cosx_pool.tile([128, E], FP16,
                                        name=f"tn_{jn}", tag="rt")
                    nc.vector.tensor_scalar(tn[:], xn_t[:], MAGIC, MAGIC,
                                            A.add, A.subtract)
                    nc.vector.tensor_tensor(xn_t[:], xn_t[:], tn[:],
                                            A.subtract)
                    nc.vector.memset(qv[:, half, :, 64:65], 1.0)
                    uv = xn_t[:].rearrange("p (h c) -> p h c", c=64)
                    nc.scalar.activation(qv[:, half, :, 0:64], uv, AF.Sin,
                                         bias=0.0, scale=2.0 * PI)

            # ============ phase A: qpn + qpT + wvo + projections =============
            with (
                tc.tile_pool(name="pspr", bufs=2, space="PSUM") as pspr,
            ):
                for t in range(8):
                    build_qpn(t)
                    # qpT for pair t (transposed layout, fp8)
                    qpt = qpT8[t]
                    xt_t = cosx_pool.tile([128, S], FP16,
                                          name=f"xt_{t}", tag="xt_in")
                    nc.sync.dma_start(xt_t[:],
                                      xt.ap()[128 * t:128 * t + 128, :])
                    rt = cosx_pool.tile([128, S], FP16,
                                        name=f"rt_{t}", tag="xrt")
                    nc.vector.tensor_scalar(rt[:], xt_t[:], MAGIC,
                                            MAGIC, A.add, A.subtract)
                    nc.vector.tensor_tensor(xt_t[:], xt_t[:], rt[:],
                                            A.subtract)
                    nc.scalar.activation(qpt[:], xt_t[:], AF.Sin,
                                         bias=0.0, scale=2.0 * PI)

                    # u = (Wk Wq^T) qp_q + Wk bq  (k-proj folded into
                    # scores; bk terms cancel under softmax shift-invariance)
                    psq = pspr.tile([128, 1024], F32, name=f"psq_{t}",
                                    tag="prj")
                    for sh in range(2):
                        ss = slice(512 * sh, 512 * sh + 512)
                        nc.tensor.matmul(psq[0:64, ss], g2_t[0:64, :],
                                         qpt[0:64, ss], start=True, stop=True)
                        nc.tensor.matmul(psq[64:128, ss], g2_t[64:128, :],
                                         qpt[64:128, ss], start=True, stop=True)
                    nc.scalar.activation(uT8[t][:], psq[:], AF.Identity,
                                         bias=bu2_t[:, 0:1], scale=1.0)


            # ===== phase B+C: attention per query-half, overlapped out-proj ==
            with (
                tc.tile_pool(name="et", bufs=3) as et_pool,
                tc.tile_pool(name="sch", bufs=2) as sch_pool,
                tc.tile_pool(name="crw", bufs=3) as crw_pool,
                tc.tile_pool(name="nrm", bufs=2) as nrm_pool,
                tc.tile_pool(name="ph4", bufs=2) as p4,
                tc.tile_pool(name="drn", bufs=6, space="DRAM") as drn_pool,
                tc.tile_pool(name="ps_s", bufs=2, space="PSUM") as ps_s,
                tc.tile_pool(name="ps_c", bufs=2, space="PSUM") as ps_c,
                tc.tile_pool(name="ps4", bufs=2, space="PSUM") as ps4,
            ):
                pending = []

                def flush_pending(fast=False):
                    for (key, craw, qs) in pending:
                        hh_ = key % 100
                        if fast:
                            # latency-critical last head: direct reciprocal
                            recd = nrm_pool.tile([1, 512], F32,
                                                 name=f"recd_{key}", tag="recd")
                            nc.vector.reciprocal(recd[:], craw[64:65, :])
                            drd = drn_pool.tile([1, 512], F32,
                                                name=f"drd_{key}", tag="drd")
                            nc.sync.dma_start(drd[:], recd[:])
                            bcd = nrm_pool.tile([64, 512], F32,
                                                name=f"bcd_{key}", tag="bcd")
                            nc.sync.dma_start(bcd[:],
                                              drd[:].broadcast_to([64, 512]))
                            t2, o2 = hh_ // 2, 64 * (hh_ % 2)
                            nc.vector.tensor_mul(ctxT[t2][o2:o2 + 64, qs],
                                                 craw[0:64, :], bcd[:])
                            continue
                        # spread denominators over 64 partitions for reciprocal
                        dr1 = drn_pool.tile([1, 512], F32,
                                            name=f"dr1_{key}", tag="dr1")
                        nc.sync.dma_start(dr1[:], craw[64:65, :])
                        den8 = nrm_pool.tile([64, 8], F32,
                                             name=f"den8_{key}", tag="den8")
                        nc.sync.dma_start(
                            den8[:],
                            dr1[:].rearrange("a (b c) -> (a b) c", c=8))
                        rec8 = nrm_pool.tile([64, 8], F32,
                                             name=f"rec8_{key}", tag="rec8")
                        nc.vector.reciprocal(rec8[:], den8[:])
                        dr2 = drn_pool.tile([1, 512], F32,
                                            name=f"dr2_{key}", tag="dr2")
                        nc.sync.dma_start(
                            dr2[:].rearrange("a (b c) -> (a b) c", c=8),
                            rec8[:])
                        bc = nrm_pool.tile([64, 512], F32,
                                           name=f"bc_{key}", tag="bc")
                        nc.sync.dma_start(bc[:], dr2[:].broadcast_to([64, 512]))
                        t2, o2 = hh_ // 2, 64 * (hh_ % 2)
                        nc.vector.tensor_mul(ctxT[t2][o2:o2 + 64, qs],
                                             craw[0:64, :], bc[:])
                    pending.clear()

                def out_proj(ic):
                    ics = slice(128 * ic, 128 * ic + 128)
                    ot = p4.tile([128, E], F32, name=f"ot_{ic}", tag="ot")
                    for nt in range(2):
                        ns = slice(512 * nt, 512 * nt + 512)
                        ops_ = ps4.tile([128, 512], F32,
                                        name=f"ops_{ic}_{nt}", tag="ops")
                        for t in range(8):
                            nc.tensor.matmul(ops_[:], ctxT[t][:, ics],
                                             wvo[t][:, ns],
                                             start=(t == 0), stop=(t == 7))
                        nc.vector.tensor_add(ot[:, ns], ops_[:], bobc_t[:, ns])
                    nc.sync.dma_start(out.ap()[ics, :], ot[:])

                for qh in range(2):
                    qs = slice(512 * qh, 512 * qh + 512)
                    for h in range(H):
                        t, hh = h // 2, h % 2
                        rows = slice(64 * hh, 64 * hh + 64)
                        ctxps = ps_c.tile([65, 512], F32,
                                          name=f"ctx_{qh}_{h}", tag="ctx")
                        ets = []
                        for m in range(8):
                            et = et_pool.tile([128, 1024], F8,
                                              name=f"et_{qh}_{h}_{m}", tag="et")
                            sp = ps_s.tile([128, 1024], F32,
                                           name=f"sp_{qh}_{h}_{m}", tag="sp")
                            for half in range(2):
                                c = 2 * m + half
                                nc.tensor.matmul(
                                    sp[:, 512 * half:512 * half + 512],
                                    qpT8[t][rows, 128 * c:128 * c + 128],
                                    uT8[t][rows, qs],
                                    start=True, stop=True)
                            if m == 4:
                                # Schraudolph exp on DVE for the last chunk
                                # pair: trims the ACT-bound exp stream; the
                                # DVE ops here gate nothing that isn't
                                # already dependent on et(7).
                                ei = sch_pool.tile([128, 1024], I32,
                                                   name=f"ei_{qh}_{h}",
                                                   tag="ei")
                                nc.vector.tensor_scalar(
                                    ei[:], sp[:], 12102203.1616 * 0.125,
                                    1064866805.0, A.mult, A.add)
                                nc.vector.tensor_copy(et[:],
                                                      ei[:].bitcast(F32))
                            else:
                                nc.scalar.activation(et[:], sp[:], AF.Exp,
                                                     bias=0.0, scale=0.125)
                            ets.append(et)
                            if m == 0:
                                flush_pending()
                            if m > 1:
                                _emit_ctx(nc, qpn, ctxps, ets[m - 2], h, m - 2)
                        _emit_ctx(nc, qpn, ctxps, ets[6], h, 6)
                        _emit_ctx(nc, qpn, ctxps, ets[7], h, 7)
                        craw = crw_pool.tile([65, 512], F32,
                                             name=f"craw_{qh}_{h}", tag="craw")
                        nc.vector.tensor_copy(craw[:], ctxps[:])
                        pending.append((100 * qh + h, craw, qs))
                        # qh0's out-projection chunks slide into qh1's
                        # ACT-bound stretch (post-throttle, PE has slack)
                        if qh == 1 and h in (8, 10, 12, 14):
                            out_proj((h - 8) // 2)
                    flush_pending(fast=(qh == 1))
                    if qh == 1:
                        for ic in range(4, 8):
                            out_proj(ic)

    return nc


def _emit_ctx(nc, qpn, ctxps, et, h, m):
    """ctx accumulation for chunk-pair m of head h (fp8 DoubleRow, K=256)."""
    qpv = qpn[m][:].rearrange("p (two h c) -> p two h c", two=2, c=65)
    ev = et[:].rearrange("p (two q) -> p two q", two=2)
    nc.tensor.matmul(ctxps[:], qpv[:, :, h, :], ev,
                     start=(m == 0), stop=(m == 7), perf_mode=DRM)


def _make_in_maps(x, theta, Wq, bq, Wk, bk, Wv, bv, Wo, bo):
    thE = (np.tile(theta, H) + PI / 2)  # per-embedding-dim phase
    g1 = (Wq @ Wk.T)          # lhsT[e',e]: u = (Wk Wq^T) qp_q
    g2 = np.concatenate([g1, g1], axis=0).astype(nf8)
    bu1 = (Wk @ bq).reshape(HD, 1)
    bu2 = np.concatenate([bu1, bu1], axis=0).astype(np.float32)
    wvo_np = np.empty((E, E), np.float32)
    for t in range(H):
        wvo_np[64 * t:64 * t + 64] = Wv @ Wo[64 * t:64 * t + 64]
    wvo_b = wvo_np.astype(nbf16)
    bob_r = (np.tile(bv, H) @ Wo + bo).reshape(1, E).astype(np.float32)

    in_maps = []
    for c in range(N_CORES):
        b, j = c // 2, c % 2
        xb = np.roll(x[b], -SQ * j, axis=0)
        yb = ((xb + thE) * (1.0 / (2.0 * PI))).astype(np.float16)
        in_maps.append(dict(
            xt=np.ascontiguousarray(yb.T),
            xn=np.ascontiguousarray(yb),
            g2=g2, wvo_in=wvo_b,
            bu2=bu2, bob_r=bob_r,
        ))
    return in_maps


def kernel(x, theta, Wq, bq, Wk, bk, Wv, bv, Wo, bo):
    x = np.asarray(x, np.float32)
    theta = np.asarray(theta, np.float32)
    Wq = np.asarray(Wq, np.float32)
    Wk = np.asarray(Wk, np.float32)
    Wv = np.asarray(Wv, np.float32)
    Wo = np.asarray(Wo, np.float32)
    bq = np.asarray(bq, np.float32)
    bk = np.asarray(bk, np.float32)
    bv = np.asarray(bv, np.float32)
    bo = np.asarray(bo, np.float32)

    nc = _build()
    _split_multiwaits(nc)

    in_maps = _make_in_maps(x, theta, Wq, bq, Wk, bk, Wv, bv, Wo, bo)

    kw = {}
    if TRACE:
        kw = dict(trace=True, trace_cores=[0])
    res = run_bass_kernel_spmd(nc, in_maps, core_ids=list(range(N_CORES)), **kw)
    global LAST_RES
    LAST_RES = res

    out = np.empty((B, S, E), np.float32)
    for c in range(N_CORES):
        b, j = c // 2, c % 2
        out[b, SQ * j:SQ * (j + 1), :] = res.results[c]["out"]
    return out


# revision 33
# speedup vs baseline: 1.1855x; 1.1855x over previous
"""Quantum multi-head attention TRN2 kernel (self-contained).

Problem: x(4,2048,1024); qp=cos(x+theta) per-head(16x64); q/k/v = qp@W*+b*
(per-head shared 64x64 weights); full softmax attention; merge heads; @Wo+bo.

Sharding: 8 cores = (batch b, seq-half j).  Each core gets the full batch-b
sequence (rolled so its 1024 query rows come first) and computes attention for
all 16 heads over its query rows, plus the final out-projection.  No
collectives; host just concatenates core outputs.

Device algorithm per core:
  host prep: y = (x + theta + pi/2)/(2pi) (fp16, both layouts);
    G = Wq@Wk^T (k-projection folded into scores; the bk terms cancel
    under softmax shift-invariance, the bq term folds into u's bias);
    wvo = blockdiag(Wv)@Wo bf16; bob = bv-row@Wo + bo.
  phase A (trig ACT table):
    qpT8 = sin2pi-range-reduce(y) -> ACT Sin -> fp8  (transposed layout)
    qpn  = same in natural layout, [qp|1] fp8 chunk-pair tiles
    u = G @ qpT8 + bu  per pair via row-grouped fp8 matmuls ->
        ACT Identity+bias -> fp8 uT8
  phase B (exp ACT table), per query-half, per head:
    scores = qpT8^T uT8   plain fp8 matmuls (K=64, full rate)
    e = exp(scores/8)     ACT -> fp8 [128,1024] = [2 chunks x 512q]
    ctx+den = qpn^T e     fp8 DoubleRow (K=256 chunk pairs) psum [65,512]
    ctxT = ctx * (1/den)  bf16, deferred normalize (spread reciprocal)
    after each query-half: its out-projection (bf16) overlaps the other
    half's ACT-bound attention.
"""
import numpy as np
import ml_dtypes

import concourse.bass as bass
import concourse.mybir as mybir
import concourse.tile as tile
from concourse.bass_utils import run_bass_kernel_spmd

F32 = mybir.dt.float32
BF16 = mybir.dt.bfloat16
FP16 = mybir.dt.float16
F8 = mybir.dt.float8e4
nbf16 = ml_dtypes.bfloat16
nf8 = ml_dtypes.float8_e4m3
PI = float(np.pi)
MAGIC = 12582912.0  # 1.5 * 2**23: forces round in the DVE's f32 pipeline
A = mybir.AluOpType
AF = mybir.ActivationFunctionType
DRM = mybir.MatmulPerfMode.DoubleRow

B, S, E = 4, 2048, 1024
H, HD = 16, 64
SQ = 1024          # query rows per core
N_CORES = 8
TRACE = False
LAST_RES = None


def _split_multiwaits(nc):
    """This container's walrus supports ONE sync-wait per instruction; split
    extras onto single-wait no-ops on the same engine (program order keeps
    semantics)."""
    counter = 0
    for f in nc.m.functions:
        for bb in f.blocks:
            new_insts = []
            for inst in bb.instructions:
                si = inst.sync_info
                if si is not None and si.on_wait and len(si.on_wait) > 1:
                    waits = list(si.on_wait)
                    si.on_wait = [waits[-1]]
                    for w in waits[:-1]:
                        counter += 1
                        new_insts.append(mybir.InstNoOp(
                            name=f"splitw-{counter}",
                            engine=inst.engine,
                            sync_info=mybir.SyncInfo(on_wait=[w], on_update=[]),
                            bass_nofuse=True,
                        ))
                new_insts.append(inst)
            bb.instructions[:] = new_insts
    return counter


def _build():
    nc = bass.Bass("TRN2", target_bir_lowering=False, debug=False)

    xt = nc.dram_tensor("xt", [E, S], FP16, kind="ExternalInput")
    xn = nc.dram_tensor("xn", [S, E], FP16, kind="ExternalInput")
    g2 = nc.dram_tensor("g2", [128, HD], F8, kind="ExternalInput")
    wvo_in = nc.dram_tensor("wvo_in", [E, E], BF16, kind="ExternalInput")
    bu2 = nc.dram_tensor("bu2", [128, 1], F32, kind="ExternalInput")
    bob_r = nc.dram_tensor("bob_r", [1, E], F32, kind="ExternalInput")
    out = nc.dram_tensor("out", [SQ, E], F32, kind="ExternalOutput")

    with tile.TileContext(nc) as tc:
        with (
            tc.tile_pool(name="persist", bufs=1) as pp,
            tc.tile_pool(name="cosx", bufs=5) as cosx_pool,
        ):
            # ---- persistent consts
            g2_t = pp.tile([128, HD], F8, name="g2_t")
            nc.sync.dma_start(g2_t[:], g2.ap())
            bu2_t = pp.tile([128, 1], F32, name="bu2_t")
            nc.sync.dma_start(bu2_t[:], bu2.ap())
            bobc_t = pp.tile([128, E], F32, name="bobc_t")
            nc.sync.dma_start(bobc_t[:], bob_r.ap().broadcast_to([128, E]))

            # persistent big arrays
            qpn = [pp.tile([128, 2 * H * 65], F8, name=f"qpn_{m}")
                   for m in range(8)]
            qpT8 = [pp.tile([128, S], F8, name=f"qpT8_{t}") for t in range(8)]
            uT8 = [pp.tile([128, SQ], F8, name=f"uT8_{t}") for t in range(8)]
            ctxT = [pp.tile([128, SQ], BF16, name=f"ctxT_{t}") for t in range(8)]
            wvo = [pp.tile([128, E], BF16, name=f"wvo_{t}") for t in range(8)]
            for t in range(8):
                nc.sync.dma_start(wvo[t][:],
                                  wvo_in.ap()[128 * t:128 * t + 128, :])

            def build_qpn(m):
                # qpn chunk-pair m: [qp|1] fp8, ctx DoubleRow stationary
                qv = qpn[m][:].rearrange("p (two h c) -> p two h c",
                                         two=2, c=65)
                for half in range(2):
                    jn = 2 * m + half
                    xn_t = cosx_pool.tile([128, E], FP16,
                                          name=f"xn_{jn}", tag="xn_in")
                    nc.sync.dma_start(xn_t[:],
                                      xn.ap()[128 * jn:128 * jn + 128, :])
                    tn = cosx_pool.tile([128, E], FP16,
                                        name=f"tn_{jn}", tag="rt")
                    nc.vector.tensor_scalar(tn[:], xn_t[:], MAGIC, MAGIC,
                                            A.add, A.subtract)
                    nc.vector.tensor_tensor(xn_t[:], xn_t[:], tn[:],
                                            A.subtract)
                    nc.vector.memset(qv[:, half, :, 64:65], 1.0)
                    uv = xn_t[:].rearrange("p (h c) -> p h c", c=64)
                    nc.scalar.activation(qv[:, half, :, 0:64], uv, AF.Sin,
                                         bias=0.0, scale=2.0 * PI)

            # ============ phase A: qpn + qpT + wvo + projections =============
            with (
                tc.tile_pool(name="pspr", bufs=2, space="PSUM") as pspr,
            ):
                for t in range(8):
                    build_qpn(t)
                    # qpT for pair t (transposed layout, fp8)
                    qpt = qpT8[t]
                    xt_t = cosx_pool.tile([128, S], FP16,
                                          name=f"xt_{t}", tag="xt_in")
                    nc.sync.dma_start(xt_t[:],
                                      xt.ap()[128 * t:128 * t + 128, :])
                    rt = cosx_pool.tile([128, S], FP16,
                                        name=f"rt_{t}", tag="xrt")
                    nc.vector.tensor_scalar(rt[:], xt_t[:], MAGIC,
                                            MAGIC, A.add, A.subtract)
                    nc.vector.tensor_tensor(xt_t[:], xt_t[:], rt[:],
                                            A.subtract)
                    nc.scalar.activation(qpt[:], xt_t[:], AF.Sin,
                                         bias=0.0, scale=2.0 * PI)

                    # u = (Wk Wq^T) qp_q + Wk bq  (k-proj folded into
                    # scores; bk terms cancel under softmax shift-invariance)
                    psq = pspr.tile([128, 1024], F32, name=f"psq_{t}",
                                    tag="prj")
                    for sh in range(2):
                        ss = slice(512 * sh, 512 * sh + 512)
                        nc.tensor.matmul(psq[0:64, ss], g2_t[0:64, :],
                                         qpt[0:64, ss], start=True, stop=True)
                        nc.tensor.matmul(psq[64:128, ss], g2_t[64:128, :],
                                         qpt[64:128, ss], start=True, stop=True)
                    nc.scalar.activation(uT8[t][:], psq[:], AF.Identity,
                                         bias=bu2_t[:, 0:1], scale=1.0)


            # ===== phase B+C: attention per query-half, overlapped out-proj ==
            with (
                tc.tile_pool(name="et", bufs=3) as et_pool,
                tc.tile_pool(name="crw", bufs=3) as crw_pool,
                tc.tile_pool(name="nrm", bufs=2) as nrm_pool,
                tc.tile_pool(name="ph4", bufs=2) as p4,
                tc.tile_pool(name="drn", bufs=6, space="DRAM") as drn_pool,
                tc.tile_pool(name="ps_s", bufs=2, space="PSUM") as ps_s,
                tc.tile_pool(name="ps_c", bufs=2, space="PSUM") as ps_c,
                tc.tile_pool(name="ps4", bufs=2, space="PSUM") as ps4,
            ):
                pending = []

                def flush_pending(fast=False):
                    for (key, craw, qs) in pending:
                        hh_ = key % 100
                        if fast:
                            # latency-critical last head: direct reciprocal
                            recd = nrm_pool.tile([1, 512], F32,
                                                 name=f"recd_{key}", tag="recd")
                            nc.vector.reciprocal(recd[:], craw[64:65, :])
                            drd = drn_pool.tile([1, 512], F32,
                                                name=f"drd_{key}", tag="drd")
                            nc.sync.dma_start(drd[:], recd[:])
                            bcd = nrm_pool.tile([64, 512], F32,
                                                name=f"bcd_{key}", tag="bcd")
                            nc.sync.dma_start(bcd[:],
                                              drd[:].broadcast_to([64, 512]))
                            t2, o2 = hh_ // 2, 64 * (hh_ % 2)
                            nc.vector.tensor_mul(ctxT[t2][o2:o2 + 64, qs],
                                                 craw[0:64, :], bcd[:])
                            continue
                        # spread denominators over 64 partitions for reciprocal
                        dr1 = drn_pool.tile([1, 512], F32,
                                            name=f"dr1_{key}", tag="dr1")
                        nc.sync.dma_start(dr1[:], craw[64:65, :])
                        den8 = nrm_pool.tile([64, 8], F32,
                                             name=f"den8_{key}", tag="den8")
                        nc.sync.dma_start(
                            den8[:],
                            dr1[:].rearrange("a (b c) -> (a b) c", c=8))
                        rec8 = nrm_pool.tile([64, 8], F32,
                                             name=f"rec8_{key}", tag="rec8")
                        nc.vector.reciprocal(rec8[:], den8[:])
                        dr2 = drn_pool.tile([1, 512], F32,
                                            name=f"dr2_{key}", tag="dr2")
                        nc.sync.dma_start(
                            dr2[:].rearrange("a (b c) -> (a b) c", c=8),
                            rec8[:])
                        bc = nrm_pool.tile([64, 512], F32,
                                           name=f"bc_{key}", tag="bc")
                        nc.sync.dma_start(bc[:], dr2[:].broadcast_to([64, 512]))
                        t2, o2 = hh_ // 2, 64 * (hh_ % 2)
                        nc.vector.tensor_mul(ctxT[t2][o2:o2 + 64, qs],
                                             craw[0:64, :], bc[:])
                    pending.clear()

                def out_proj(ic):
                    ics = slice(128 * ic, 128 * ic + 128)
                    ot = p4.tile([128, E], F32, name=f"ot_{ic}", tag="ot")
                    for nt in range(2):
                        ns = slice(512 * nt, 512 * nt + 512)
                        ops_ = ps4.tile([128, 512], F32,
                                        name=f"ops_{ic}_{nt}", tag="ops")
                        for t in range(8):
                            nc.tensor.matmul(ops_[:], ctxT[t][:, ics],
                                             wvo[t][:, ns],
                                             start=(t == 0), stop=(t == 7))
                        nc.vector.tensor_add(ot[:, ns], ops_[:], bobc_t[:, ns])
                    nc.sync.dma_start(out.ap()[ics, :], ot[:])

                for qh in range(2):
                    qs = slice(512 * qh, 512 * qh + 512)
                    for h in range(H):
                        t, hh = h // 2, h % 2
                        rows = slice(64 * hh, 64 * hh + 64)
                        ctxps = ps_c.tile([65, 512], F32,
                                          name=f"ctx_{qh}_{h}", tag="ctx")
                        ets = []
                        for m in range(8):
                            et = et_pool.tile([128, 1024], F8,
                                              name=f"et_{qh}_{h}_{m}", tag="et")
                            sp = ps_s.tile([128, 1024], F32,
                                           name=f"sp_{qh}_{h}_{m}", tag="sp")
                            for half in range(2):
                                c = 2 * m + half
                                nc.tensor.matmul(
                                    sp[:, 512 * half:512 * half + 512],
                                    qpT8[t][rows, 128 * c:128 * c + 128],
                                    uT8[t][rows, qs],
                                    start=True, stop=True)
                            nc.scalar.activation(et[:], sp[:], AF.Exp,
                                                 bias=0.0, scale=0.125)
                            ets.append(et)
                            if m == 0:
                                flush_pending()
                            if m > 1:
                                _emit_ctx(nc, qpn, ctxps, ets[m - 2], h, m - 2)
                        _emit_ctx(nc, qpn, ctxps, ets[6], h, 6)
                        _emit_ctx(nc, qpn, ctxps, ets[7], h, 7)
                        craw = crw_pool.tile([65, 512], F32,
                                             name=f"craw_{qh}_{h}", tag="craw")
                        nc.vector.tensor_copy(craw[:], ctxps[:])
                        pending.append((100 * qh + h, craw, qs))
                        # qh0's out-projection chunks slide into qh1's
                        # ACT-bound stretch (post-throttle, PE has slack)
                        if qh == 1 and h in (8, 10, 12, 14):
                            out_proj((h - 8) // 2)
                    flush_pending(fast=(qh == 1))
                    if qh == 1:
                        for ic in range(4, 8):
                            out_proj(ic)

    return nc


def _emit_ctx(nc, qpn, ctxps, et, h, m):
    """ctx accumulation for chunk-pair m of head h (fp8 DoubleRow, K=256)."""
    qpv = qpn[m][:].rearrange("p (two h c) -> p two h c", two=2, c=65)
    ev = et[:].rearrange("p (two q) -> p two q", two=2)
    nc.tensor.matmul(ctxps[:], qpv[:, :, h, :], ev,
                     start=(m == 0), stop=(m == 7), perf_mode=DRM)


def _make_in_maps(x, theta, Wq, bq, Wk, bk, Wv, bv, Wo, bo):
    thE = (np.tile(theta, H) + PI / 2)  # per-embedding-dim phase
    g1 = (Wq @ Wk.T)          # lhsT[e',e]: u = (Wk Wq^T) qp_q
    g2 = np.concatenate([g1, g1], axis=0).astype(nf8)
    bu1 = (Wk @ bq).reshape(HD, 1)
    bu2 = np.concatenate([bu1, bu1], axis=0).astype(np.float32)
    wvo_np = np.empty((E, E), np.float32)
    for t in range(H):
        wvo_np[64 * t:64 * t + 64] = Wv @ Wo[64 * t:64 * t + 64]
    wvo_b = wvo_np.astype(nbf16)
    bob_r = (np.tile(bv, H) @ Wo + bo).reshape(1, E).astype(np.float32)

    in_maps = []
    for c in range(N_CORES):
        b, j = c // 2, c % 2
        xb = np.roll(x[b], -SQ * j, axis=0)
        yb = ((xb + thE) * (1.0 / (2.0 * PI))).astype(np.float16)
        in_maps.append(dict(
            xt=np.ascontiguousarray(yb.T),
            xn=np.ascontiguousarray(yb),
            g2=g2, wvo_in=wvo_b,
            bu2=bu2, bob_r=bob_r,
        ))
    return in_maps


def kernel(x, theta, Wq, bq, Wk, bk, Wv, bv, Wo, bo):
    x = np.asarray(x, np.float32)
    theta = np.asarray(theta, np.float32)
    Wq = np.asarray(Wq, np.float32)
    Wk = np.asarray(Wk, np.float32)
    Wv = np.asarray(Wv, np.float32)
    Wo = np.asarray(Wo, np.float32)
    bq = np.asarray(bq, np.float32)
    bk = np.asarray(bk, np.float32)
    bv = np.asarray(bv, np.float32)
    bo = np.asarray(bo, np.float32)

    nc = _build()
    _split_multiwaits(nc)

    in_maps = _make_in_maps(x, theta, Wq, bq, Wk, bk, Wv, bv, Wo, bo)

    kw = {}
    if TRACE:
        kw = dict(trace=True, trace_cores=[0])
    res = run_bass_kernel_spmd(nc, in_maps, core_ids=list(range(N_CORES)), **kw)
    global LAST_RES
    LAST_RES = res

    out = np.empty((B, S, E), np.float32)
    for c in range(N_CORES):
        b, j = c // 2, c % 2
        out[b, SQ * j:SQ * (j + 1), :] = res.results[c]["out"]
    return out
